# revision 19
# baseline (speedup 1.0000x reference)
"""Trainium2 Bass kernel for a 7-layer stacked LSTM decoder cell (single step).

Strategy (8 NeuronCores, tensor parallel):
  - Column-parallel shard of every layer's W_ih / W_hh along the 4H gate
    output dim: core j owns rows {q*2048 + j*256 .. q*2048 + (j+1)*256} for
    each gate q in (i, f, g, o)  ->  1024 gate rows per core per matrix.
  - Weights are pre-transposed on the host to [H, 1024] (contraction dim on
    partitions) and split into bf16 hi/lo pairs (hi + lo == fp32 to ~1e-5
    relative), packed in DMA-friendly [128, 32768] chunks.
  - PE computes each matvec as rhs-streaming matmuls: lhsT = x column pair
    (hi at PE col-group 0, lo at col-group 32, running concurrently),
    rhs = W^T tiles [128, 512].  PSUM accumulates over 16 K-tiles and both
    weight halves; gate row 0 + row 32 are combined on DVE.
  - Only the layer-input chain x_{l+1} = h_l (+ h_{l-1}) is serial; all seven
    W_hh @ h_l products use the *input* states and are scheduled as soon as
    their weights stream in.  Hidden-state slices are exchanged between
    layers with an 8-core AllGather of [1, 256] f32, transposed back to
    column-major via a PE identity-matmul transpose.
"""

import sys

sys.path.insert(0, "/opt/trn_rl_repo")

import numpy as np
import ml_dtypes

H = 2048
L = 7
NCORES = 8
SL = H // NCORES          # 256: per-core slice of each gate / of h
GP = 4 * SL               # 1024: gate rows per core per matrix
KT = H // 128             # 16 k-tiles
CHUNK_COLS = KT * 2 * GP  # 32768 packed cols per (layer, matrix)
NCHUNK = 2                     # DMA chunks per matrix
CH_COLS = CHUNK_COLS // NCHUNK # 8192 cols = 4 k-tiles per chunk (2 MiB)
KT_PER_CH = KT // NCHUNK

BF16 = ml_dtypes.bfloat16

LAST_RESULT = None        # stashed BassKernelResults for test harnesses


def _build_program():
    import concourse.bacc as bacc
    import concourse.mybir as mybir
    import concourse.tile as tile

    F32 = mybir.dt.float32
    BF = mybir.dt.bfloat16
    ACT = mybir.ActivationFunctionType
    AX = mybir.AxisListType

    nc = bacc.Bacc(None)

    # ---- DRAM I/O ----
    w_ins = {}
    for l in range(1, L + 1):
        for c in range(NCHUNK):
            w_ins[("whh", l, c)] = nc.dram_tensor(f"whh{l}c{c}", [128, CH_COLS], BF, kind="ExternalInput")
    for l in range(2, L + 1):
        for c in range(NCHUNK):
            w_ins[("wih", l, c)] = nc.dram_tensor(f"wih{l}c{c}", [128, CH_COLS], BF, kind="ExternalInput")
    wih1_in = nc.dram_tensor("wih1", [2, 2 * GP], BF, kind="ExternalInput")
    x12_in = nc.dram_tensor("x12", [2, 2], BF, kind="ExternalInput")
    hcols_in = nc.dram_tensor("hcols", [128, L * 2 * KT], BF, kind="ExternalInput")
    cvec_in = nc.dram_tensor("cvec", [1, L * SL], F32, kind="ExternalInput")
    bvec_in = nc.dram_tensor("bvec", [1, L * GP], F32, kind="ExternalInput")
    woutT_in = nc.dram_tensor("woutT", [128, KT * 4], BF, kind="ExternalInput")
    bout_in = nc.dram_tensor("bout", [1, 2], F32, kind="ExternalInput")
    npn_in = nc.dram_tensor("npn", [1, 2], F32, kind="ExternalInput")
    ident_in = nc.dram_tensor("ident", [16, 16], F32, kind="ExternalInput")

    probs_out = nc.dram_tensor("probs", [1, 2], F32, kind="ExternalOutput")
    outh_out = nc.dram_tensor("outh", [L, SL], F32, kind="ExternalOutput")
    outc_out = nc.dram_tensor("outc", [L, SL], F32, kind="ExternalOutput")

    with tile.TileContext(nc) as tc:
        with (
            tc.tile_pool(name="wpool", bufs=5) as wpool,
            tc.tile_pool(name="small", bufs=1) as small,
            tc.tile_pool(name="work", bufs=1) as work,
            tc.tile_pool(name="gps", bufs=3, space="PSUM") as gps,
            tc.tile_pool(name="tps", bufs=1, space="PSUM") as tps,
            tc.tile_pool(name="dram", bufs=1, space="DRAM") as dram,
        ):
            # ---- small input loads (ACT HWDGE ring, ahead of weight stream) ----
            hcols_sb = small.tile([128, L * 2 * KT], BF, name="hcols_sb")
            nc.scalar.dma_start(hcols_sb[:], hcols_in[:, :])
            wih1_sb = small.tile([2, 2 * GP], BF, name="wih1_sb")
            nc.scalar.dma_start(wih1_sb[:], wih1_in[:, :])
            x12_sb = small.tile([2, 2], BF, name="x12_sb")
            nc.scalar.dma_start(x12_sb[:], x12_in[:, :])
            cvec_tiles = {}
            for l in range(1, L + 1):
                ct = small.tile([1, SL], F32, name=f"cvec{l}_sb", tag="cvec", bufs=2)
                nc.scalar.dma_start(ct[:], cvec_in[:, (l - 1) * SL: l * SL])
                cvec_tiles[l] = ct
            bvec_tiles = {}
            for l in range(1, L + 1):
                bt = small.tile([1, GP], F32, name=f"bvec{l}_sb", tag="bvec", bufs=2)
                nc.scalar.dma_start(bt[:], bvec_in[:, (l - 1) * GP: l * GP])
                bvec_tiles[l] = bt
            woutT_sb = small.tile([128, KT * 4], BF, name="woutT_sb")
            nc.scalar.dma_start(woutT_sb[:], woutT_in[:, :])
            bout_sb = small.tile([1, 2], F32, name="bout_sb")
            nc.scalar.dma_start(bout_sb[:], bout_in[:, :])
            npn_sb = small.tile([1, 2], F32, name="npn_sb")
            nc.scalar.dma_start(npn_sb[:], npn_in[:, :])
            ident_sb = small.tile([16, 16], F32, name="ident_sb")
            nc.scalar.dma_start(ident_sb[:], ident_in[:, :])

            h_tiles = {}      # layer -> [1, SL] f32 output slice
            dram_tiles = {}
            xcol_tiles = {}   # layer -> [128, 2*KT] bf16 input cols (hi even, lo odd)

            def mm_quad(psum, lhsT_hi, lhsT_lo, rhs_hi, rhs_lo, first, last):
                """x(hi,lo) * W(hi,lo): hi products to psum row 0 (col grp 0),
                lo products to row 32 (col grp 32), running concurrently."""
                nc.tensor.matmul(psum[0:1, :], lhsT_hi, rhs_hi, start=first, stop=False)
                nc.tensor.matmul(psum[32:33, :], lhsT_lo, rhs_hi, start=first, stop=False,
                                 tile_position=(0, 32))
                nc.tensor.matmul(psum[0:1, :], lhsT_hi, rhs_lo, start=False, stop=last)
                nc.tensor.matmul(psum[32:33, :], lhsT_lo, rhs_lo, start=False, stop=last,
                                 tile_position=(0, 32))

            def issue_matrix(psum, wchunks, lx, first, last):
                """128 matmuls: one [1024-row] matrix against x cols lx (callable
                t -> (hi_ap, lo_ap)); gate col block ns lands in psum[:, ns*512:]."""
                for t in range(KT):
                    hi, lo = lx(t)
                    wchunk = wchunks[t // KT_PER_CH]
                    base = (t % KT_PER_CH) * 2 * GP
                    for ns in (0, 1):
                        ps = psum[:, ns * 512: (ns + 1) * 512]
                        rhs_hi = wchunk[:, base + ns * 512: base + (ns + 1) * 512]
                        rhs_lo = wchunk[:, base + GP + ns * 512: base + GP + (ns + 1) * 512]
                        mm_quad(ps, hi, lo, rhs_hi, rhs_lo,
                                first and t == 0, last and t == KT - 1)

            def hcol_pair(l):
                def lx(t):
                    base = (l - 1) * 2 * KT + 2 * t
                    return hcols_sb[:, base: base + 1], hcols_sb[:, base + 1: base + 2]
                return lx

            def xcol_pair(l):
                xc = xcol_tiles[l]
                def lx(t):
                    return xc[:, 2 * t: 2 * t + 1], xc[:, 2 * t + 1: 2 * t + 2]
                return lx

            def cell_and_handoff(l, psum):
                """Gate nonlinearities + LSTM cell update for layer l, then
                AllGather of the next layer's input and rebuild of x columns."""
                # g = row0 + row32 (+ bias)
                gbuf = work.tile([1, GP], F32, name="gbuf", tag="gbuf", bufs=1)
                nc.scalar.activation(gbuf[:, :], psum[32:33, :], ACT.Copy)
                nc.vector.tensor_add(gbuf[:, :], psum[0:1, :], gbuf[:, :])
                nc.vector.tensor_add(gbuf[:, :], gbuf[:, :], bvec_tiles[l][:, :])
                # nonlinearities: [i(256) | f(256) | g(256) | o(256)]
                sgif = work.tile([1, 512], F32, name="sgif", tag="sgif", bufs=1)
                nc.scalar.activation(sgif[:, :], gbuf[:, 0:512], ACT.Sigmoid)
                tg = work.tile([1, SL], F32, name="tg", tag="tg", bufs=1)
                nc.scalar.activation(tg[:, :], gbuf[:, 512:768], ACT.Tanh)
                so = work.tile([1, SL], F32, name="so", tag="so", bufs=1)
                nc.scalar.activation(so[:, :], gbuf[:, 768:1024], ACT.Sigmoid)
                # c2 = sig(f)*c + sig(i)*tanh(g)
                t1 = work.tile([1, SL], F32, name="t1", tag="t1", bufs=1)
                nc.vector.tensor_mul(t1[:, :], sgif[:, 256:512], cvec_tiles[l][:, :])
                t2 = work.tile([1, SL], F32, name="t2", tag="t2", bufs=1)
                nc.vector.tensor_mul(t2[:, :], sgif[:, 0:256], tg[:, :])
                c2 = work.tile([1, SL], F32, name="c2", tag="c2", bufs=1)
                nc.vector.tensor_add(c2[:, :], t1[:, :], t2[:, :])
                # h2 = sig(o)*tanh(c2)
                tc2 = work.tile([1, SL], F32, name="tc2", tag="tc2", bufs=1)
                nc.scalar.activation(tc2[:, :], c2[:, :], ACT.Tanh)
                h2 = work.tile([1, SL], F32, name=f"h2_{l}")
                nc.vector.tensor_mul(h2[:, :], so[:, :], tc2[:, :])
                h_tiles[l] = h2
                # outputs
                nc.gpsimd.dma_start(outh_out[l - 1: l, :], h2[:, :])
                nc.gpsimd.dma_start(outc_out[l - 1: l, :], c2[:, :])
                # next-layer input slice: s = h_l (+ h_{l-1} residual for l >= 2)
                s = work.tile([1, SL], F32, name=f"s_{l}")
                if 2 <= l <= L - 1:
                    nc.vector.tensor_add(s[:, :], h2[:, :], h_tiles[l - 1][:, :])
                else:
                    nc.vector.tensor_copy(s[:, :], h2[:, :])
                # AllGather s -> full [1, 2048] as 16x128 rows
                agin = dram.tile([1, SL], F32, name=f"agin{l}")
                agout = dram.tile([16, 128], F32, name=f"agout{l}")
                dram_tiles[f"agout{l + 1}"] = agout
                nc.scalar.dma_start(agin[:], s[:, :])
                nc.gpsimd.collective_compute(
                    "AllGather",
                    mybir.AluOpType.bypass,
                    replica_groups=[list(range(NCORES))],
                    ins=[agin[:].opt()],
                    outs=[agout[:].opt()],
                )
                x16 = work.tile([16, 128], F32, name="x16", tag="x16", bufs=1)
                nc.scalar.dma_start(x16[:], agout[:])
                # transpose to [128, 16] and split into bf16 hi/lo columns
                psT = tps.tile([128, 16], F32, name="psT", tag="psT")
                nc.tensor.transpose(psT[:, :], x16[:, :], ident_sb[:, :])
                xc = work.tile([128, 2 * KT], BF, name=f"xcol_{l + 1}", tag="xcol", bufs=3)
                xf = work.tile([128, KT], F32, name="xf", tag="xf", bufs=1)
                nc.scalar.activation(xc[:, 0: 2 * KT: 2], psT[:, :], ACT.Copy)   # hi (even)
                nc.scalar.activation(xf[:, :], xc[:, 0: 2 * KT: 2], ACT.Copy)    # hi -> f32
                xlo = work.tile([128, KT], F32, name="xlo", tag="xlo", bufs=1)
                nc.vector.tensor_sub(xlo[:, :], psT[:, :], xf[:, :])
                nc.scalar.activation(xc[:, 1: 2 * KT: 2], xlo[:, :], ACT.Copy)   # lo (odd)
                xcol_tiles[l + 1] = xc

            # ================= main pipeline =================
            def load_chunk(kind, l, c):
                wt = wpool.tile([128, CH_COLS], BF, name=f"{kind}{l}_sb{c}", tag="w")
                nc.sync.dma_start(wt[:], w_ins[(kind, l, c)][:, :])
                return wt

            def load_matrix(kind, l):
                return [load_chunk(kind, l, c) for c in range(NCHUNK)]

            psums = {}

            def new_psum(l):
                ps = gps.tile([64, 1024], F32, name=f"ps{l}", tag="gps")
                psums[l] = ps
                return ps

            # PE order per layer: [AG_{l-1} lands] -> W_ih @ x_l (weights long
            # resident) -> W_hh @ h_{l+1} (riding the DMA stream; fills the PE
            # while the next AllGather is in flight).
            new_psum(1)
            issue_matrix(psums[1], load_matrix("whh", 1), hcol_pair(1), first=True, last=False)
            for ns in (0, 1):
                ps = psums[1][:, ns * 512: (ns + 1) * 512]
                rhs_hi = wih1_sb[0:2, ns * 512: (ns + 1) * 512]
                rhs_lo = wih1_sb[0:2, GP + ns * 512: GP + (ns + 1) * 512]
                mm_quad(ps, x12_sb[0:2, 0:1], x12_sb[0:2, 1:2], rhs_hi, rhs_lo, False, True)
            new_psum(2)
            issue_matrix(psums[2], load_matrix("whh", 2), hcol_pair(2), first=True, last=False)

            for l in range(2, L + 1):
                # previous layer cell + AllGather + x-column rebuild
                cell_and_handoff(l - 1, psums[l - 1])
                if l < L:
                    # chain-dependent W_ih @ x_l, then next layer's W_hh @ h
                    # (chain-independent, fills the PE during the next AllGather)
                    issue_matrix(psums[l], load_matrix("wih", l), xcol_pair(l), first=False, last=True)
                    new_psum(l + 1)
                    if l + 1 == L:
                        # Final layer: interleave the two matrices' chunk DMAs so
                        # the last weight bytes land as early as possible.
                        whhL, wihL = [], []
                        for c in range(NCHUNK):
                            whhL.append(load_chunk("whh", L, c))
                            wihL.append(load_chunk("wih", L, c))
                        issue_matrix(psums[L], whhL, hcol_pair(L), first=True, last=False)
                    else:
                        issue_matrix(psums[l + 1], load_matrix("whh", l + 1), hcol_pair(l + 1),
                                     first=True, last=False)
                else:
                    issue_matrix(psums[L], wihL, xcol_pair(L), first=False, last=True)

            # final layer cell + gather of h7
            cell_and_handoff(L, psums[L])

            # ---- output projection + softmax (every core computes it) ----
            # probs tolerates bf16: single-precision x and W, 16 matmuls into
            # one PSUM row, then softmax2(z) = sigmoid(+/-(z0-z1)).
            xc7 = xcol_tiles[L + 1]
            psP = gps.tile([1, 2], F32, name="psP", tag="psP", bufs=1)
            for t in range(KT):
                nc.tensor.matmul(psP[0:1, :], xc7[:, 2 * t: 2 * t + 1],
                                 woutT_sb[:, 4 * t: 4 * t + 2],
                                 start=t == 0, stop=t == KT - 1)
            z = work.tile([1, 2], F32, name="z")
            nc.vector.tensor_add(z[:, :], psP[0:1, :], bout_sb[:, :])
            dz = work.tile([1, 1], F32, name="dz")
            nc.vector.tensor_sub(dz[:, :], z[:, 0:1], z[:, 1:2])
            dd = work.tile([1, 2], F32, name="dd")
            nc.vector.tensor_scalar_mul(dd[:, :], npn_sb[:, :], dz[:, :])
            pr = work.tile([1, 2], F32, name="pr")
            nc.scalar.activation(pr[:, :], dd[:, :], ACT.Sigmoid)
            nc.scalar.dma_start(probs_out[:, :], pr[:, :])

    nc.finalize()
    return nc


def _split_bf16(a):
    hi = a.astype(BF16)
    lo = (a - hi.astype(np.float32)).astype(BF16)
    return hi, lo


def _pack_wt(w_shard_t):
    """[2048, 1024] fp32 W^T shard -> [128, 32768] bf16 packed chunk
    (col = t*2048 + half*1024 + n)."""
    hi, lo = _split_bf16(np.ascontiguousarray(w_shard_t))
    arr = np.stack([hi.reshape(KT, 128, GP), lo.reshape(KT, 128, GP)], axis=2)
    return np.ascontiguousarray(arr.transpose(1, 0, 2, 3)).reshape(128, CHUNK_COLS)


def _hcol_pack(h):
    """[2048] fp32 -> [128, 32] bf16 (hi even cols, lo odd cols)."""
    hi, lo = _split_bf16(h)
    out = np.empty((128, 2 * KT), dtype=BF16)
    out[:, 0::2] = hi.reshape(KT, 128).T
    out[:, 1::2] = lo.reshape(KT, 128).T
    return out


_ROWS = [
    np.concatenate([np.arange(q * H + j * SL, q * H + (j + 1) * SL) for q in range(4)])
    for j in range(NCORES)
]


def kernel(x, h1, c1, h2, c2, h3, c3, h4, c4, h5, c5, h6, c6, h7, c7,
           Wih1, Whh1, bih1, bhh1, Wih, Whh, bih, bhh, Wout, bout):
    global LAST_RESULT
    from concourse.bass_utils import run_bass_kernel_spmd

    x = np.asarray(x, dtype=np.float32)
    hs = [np.asarray(a, dtype=np.float32) for a in (h1, h2, h3, h4, h5, h6, h7)]
    cs = [np.asarray(a, dtype=np.float32) for a in (c1, c2, c3, c4, c5, c6, c7)]
    Wih1 = np.asarray(Wih1, dtype=np.float32)
    Whh1 = np.asarray(Whh1, dtype=np.float32)
    Wih = np.asarray(Wih, dtype=np.float32)
    Whh = np.asarray(Whh, dtype=np.float32)
    bsum = [np.asarray(bih1, np.float32) + np.asarray(bhh1, np.float32)] + [
        np.asarray(bih, np.float32)[k] + np.asarray(bhh, np.float32)[k] for k in range(6)
    ]
    Wout = np.asarray(Wout, dtype=np.float32)
    bout = np.asarray(bout, dtype=np.float32).reshape(1, 2)

    ident = np.eye(16, dtype=np.float32)
    xhi, xlo = _split_bf16(x)
    x12 = np.stack([xhi, xlo], axis=1)  # [2, 2] (hi col 0, lo col 1)

    in_maps = []
    for j in range(NCORES):
        rows = _ROWS[j]
        m = {"ident": ident, "x12": x12, "bout": bout,
             "npn": np.array([[1.0, -1.0]], dtype=np.float32)}
        ncol = CHUNK_COLS // NCHUNK
        for l in range(1, L + 1):
            Wh = Whh1 if l == 1 else Whh[l - 2]
            p = _pack_wt(Wh[rows, :].T)
            for c in range(NCHUNK):
                m[f"whh{l}c{c}"] = np.ascontiguousarray(p[:, c * ncol:(c + 1) * ncol])
            if l >= 2:
                p = _pack_wt(Wih[l - 2][rows, :].T)
                for c in range(NCHUNK):
                    m[f"wih{l}c{c}"] = np.ascontiguousarray(p[:, c * ncol:(c + 1) * ncol])
        w1hi, w1lo = _split_bf16(np.ascontiguousarray(Wih1[rows, :].T))  # [2, 1024]
        m["wih1"] = np.concatenate([w1hi, w1lo], axis=1)  # [2, 2048]
        m["hcols"] = np.concatenate([_hcol_pack(hs[l][0]) for l in range(L)], axis=1)
        m["cvec"] = np.concatenate([cs[l][0, rows[:SL]] for l in range(L)]).reshape(1, -1)
        m["bvec"] = np.concatenate([bsum[l][rows] for l in range(L)]).reshape(1, -1)
        wthi, wtlo = _split_bf16(np.ascontiguousarray(Wout.T))  # [2048, 2]
        wpack = np.stack([wthi.reshape(KT, 128, 2), wtlo.reshape(KT, 128, 2)], axis=2)
        m["woutT"] = np.ascontiguousarray(wpack.transpose(1, 0, 2, 3)).reshape(128, KT * 4)
        in_maps.append(m)

    nc = _build_program()
    import os
    trace = bool(os.environ.get("BASS_TRACE"))
    res = run_bass_kernel_spmd(nc, in_maps, list(range(NCORES)), trace=trace)
    LAST_RESULT = res

    probs = np.asarray(res.results[0]["probs"], dtype=np.float32)
    out = [probs]
    for l in range(L):
        hv = np.concatenate([res.results[j]["outh"][l] for j in range(NCORES)]).reshape(1, H)
        cv = np.concatenate([res.results[j]["outc"][l] for j in range(NCORES)]).reshape(1, H)
        out.append(hv.astype(np.float32))
        out.append(cv.astype(np.float32))
    return tuple(out)


# revision 20
# speedup vs baseline: 1.1346x; 1.1346x over previous
"""Trainium2 Bass kernel for a 7-layer stacked LSTM decoder cell (single step).

Strategy (8 NeuronCores, tensor parallel):
  - Column-parallel shard of every layer's W_ih / W_hh along the 4H gate
    output dim: core j owns rows {q*2048 + j*256 .. q*2048 + (j+1)*256} for
    each gate q in (i, f, g, o)  ->  1024 gate rows per core per matrix.
  - Weights are pre-transposed on the host to [H, 1024] (contraction dim on
    partitions) and split into bf16 hi/lo pairs (hi + lo == fp32 to ~1e-5
    relative), packed in DMA-friendly [128, 32768] chunks.
  - PE computes each matvec as rhs-streaming matmuls: lhsT = x column pair
    (hi at PE col-group 0, lo at col-group 32, running concurrently),
    rhs = W^T tiles [128, 512].  PSUM accumulates over 16 K-tiles and both
    weight halves; gate row 0 + row 32 are combined on DVE.
  - Only the layer-input chain x_{l+1} = h_l (+ h_{l-1}) is serial; all seven
    W_hh @ h_l products use the *input* states and are scheduled as soon as
    their weights stream in.  Hidden-state slices are exchanged between
    layers with an 8-core AllGather of [1, 256] f32, transposed back to
    column-major via a PE identity-matmul transpose.
"""

import sys

sys.path.insert(0, "/opt/trn_rl_repo")

import numpy as np
import ml_dtypes

H = 2048
L = 7
NCORES = 8
SL = H // NCORES          # 256: per-core slice of each gate / of h
GP = 4 * SL               # 1024: gate rows per core per matrix
KT = H // 128             # 16 k-tiles
CHUNK_COLS = KT * 2 * GP  # 32768 packed cols per (layer, matrix)
NCHUNK = 4                     # DMA chunks per matrix
CH_COLS = CHUNK_COLS // NCHUNK # 8192 cols = 4 k-tiles per chunk (2 MiB)
KT_PER_CH = KT // NCHUNK

BF16 = ml_dtypes.bfloat16

LAST_RESULT = None        # stashed BassKernelResults for test harnesses


def _build_program():
    import concourse.bacc as bacc
    import concourse.mybir as mybir
    import concourse.tile as tile

    F32 = mybir.dt.float32
    BF = mybir.dt.bfloat16
    ACT = mybir.ActivationFunctionType
    AX = mybir.AxisListType

    nc = bacc.Bacc(None)

    # ---- DRAM I/O ----
    w_ins = {}
    for l in range(1, L + 1):
        for c in range(NCHUNK):
            w_ins[("whh", l, c)] = nc.dram_tensor(f"whh{l}c{c}", [128, CH_COLS], BF, kind="ExternalInput")
    for l in range(2, L + 1):
        for c in range(NCHUNK):
            w_ins[("wih", l, c)] = nc.dram_tensor(f"wih{l}c{c}", [128, CH_COLS], BF, kind="ExternalInput")
    wih1_in = nc.dram_tensor("wih1", [2, 2 * GP], BF, kind="ExternalInput")
    x12_in = nc.dram_tensor("x12", [2, 2], BF, kind="ExternalInput")
    hcols_in = nc.dram_tensor("hcols", [128, L * 2 * KT], BF, kind="ExternalInput")
    cvec_in = nc.dram_tensor("cvec", [1, L * SL], F32, kind="ExternalInput")
    bvec_in = nc.dram_tensor("bvec", [1, L * GP], F32, kind="ExternalInput")
    woutT_in = nc.dram_tensor("woutT", [128, KT * 4], BF, kind="ExternalInput")
    bout_in = nc.dram_tensor("bout", [1, 2], F32, kind="ExternalInput")
    npn_in = nc.dram_tensor("npn", [1, 2], F32, kind="ExternalInput")
    ident_in = nc.dram_tensor("ident", [16, 16], F32, kind="ExternalInput")

    probs_out = nc.dram_tensor("probs", [1, 2], F32, kind="ExternalOutput")
    outh_out = nc.dram_tensor("outh", [L, SL], F32, kind="ExternalOutput")
    outc_out = nc.dram_tensor("outc", [L, SL], F32, kind="ExternalOutput")

    with tile.TileContext(nc) as tc:
        with (
            tc.tile_pool(name="wpool", bufs=10) as wpool,
            tc.tile_pool(name="small", bufs=1) as small,
            tc.tile_pool(name="work", bufs=1) as work,
            tc.tile_pool(name="gps", bufs=3, space="PSUM") as gps,
            tc.tile_pool(name="tps", bufs=1, space="PSUM") as tps,
            tc.tile_pool(name="dram", bufs=1, space="DRAM") as dram,
        ):
            # ---- small input loads (ACT HWDGE ring, ahead of weight stream) ----
            hcols_sb = small.tile([128, L * 2 * KT], BF, name="hcols_sb")
            nc.scalar.dma_start(hcols_sb[:], hcols_in[:, :])
            wih1_sb = small.tile([2, 2 * GP], BF, name="wih1_sb")
            nc.scalar.dma_start(wih1_sb[:], wih1_in[:, :])
            x12_sb = small.tile([2, 2], BF, name="x12_sb")
            nc.scalar.dma_start(x12_sb[:], x12_in[:, :])
            cvec_tiles = {}
            for l in range(1, L + 1):
                ct = small.tile([1, SL], F32, name=f"cvec{l}_sb", tag="cvec", bufs=2)
                nc.scalar.dma_start(ct[:], cvec_in[:, (l - 1) * SL: l * SL])
                cvec_tiles[l] = ct
            bvec_tiles = {}
            for l in range(1, L + 1):
                bt = small.tile([1, GP], F32, name=f"bvec{l}_sb", tag="bvec", bufs=2)
                nc.scalar.dma_start(bt[:], bvec_in[:, (l - 1) * GP: l * GP])
                bvec_tiles[l] = bt
            woutT_sb = small.tile([128, KT * 4], BF, name="woutT_sb")
            nc.scalar.dma_start(woutT_sb[:], woutT_in[:, :])
            bout_sb = small.tile([1, 2], F32, name="bout_sb")
            nc.scalar.dma_start(bout_sb[:], bout_in[:, :])
            npn_sb = small.tile([1, 2], F32, name="npn_sb")
            nc.scalar.dma_start(npn_sb[:], npn_in[:, :])
            ident_sb = small.tile([16, 16], F32, name="ident_sb")
            nc.scalar.dma_start(ident_sb[:], ident_in[:, :])

            h_tiles = {}      # layer -> [1, SL] f32 output slice
            dram_tiles = {}
            xcol_tiles = {}   # layer -> [128, 2*KT] bf16 input cols (hi even, lo odd)

            def mm_quad(psum, lhsT_hi, lhsT_lo, rhs_hi, rhs_lo, first, last):
                """x(hi,lo) * W(hi,lo): hi products to psum row 0 (col grp 0),
                lo products to row 32 (col grp 32), running concurrently."""
                nc.tensor.matmul(psum[0:1, :], lhsT_hi, rhs_hi, start=first, stop=False)
                nc.tensor.matmul(psum[32:33, :], lhsT_lo, rhs_hi, start=first, stop=False,
                                 tile_position=(0, 32))
                nc.tensor.matmul(psum[0:1, :], lhsT_hi, rhs_lo, start=False, stop=last)
                nc.tensor.matmul(psum[32:33, :], lhsT_lo, rhs_lo, start=False, stop=last,
                                 tile_position=(0, 32))

            def issue_matrix(psum, wchunks, lx, first, last):
                """128 matmuls: one [1024-row] matrix against x cols lx (callable
                t -> (hi_ap, lo_ap)); gate col block ns lands in psum[:, ns*512:]."""
                for t in range(KT):
                    hi, lo = lx(t)
                    wchunk = wchunks[t // KT_PER_CH]
                    base = (t % KT_PER_CH) * 2 * GP
                    for ns in (0, 1):
                        ps = psum[:, ns * 512: (ns + 1) * 512]
                        rhs_hi = wchunk[:, base + ns * 512: base + (ns + 1) * 512]
                        rhs_lo = wchunk[:, base + GP + ns * 512: base + GP + (ns + 1) * 512]
                        mm_quad(ps, hi, lo, rhs_hi, rhs_lo,
                                first and t == 0, last and t == KT - 1)

            def hcol_pair(l):
                def lx(t):
                    base = (l - 1) * 2 * KT + 2 * t
                    return hcols_sb[:, base: base + 1], hcols_sb[:, base + 1: base + 2]
                return lx

            def xcol_pair(l):
                xc = xcol_tiles[l]
                def lx(t):
                    return xc[:, 2 * t: 2 * t + 1], xc[:, 2 * t + 1: 2 * t + 2]
                return lx

            def cell_and_handoff(l, psum):
                """Gate nonlinearities + LSTM cell update for layer l, then
                AllGather of the next layer's input and rebuild of x columns."""
                # g = row0 + row32 (+ bias)
                gbuf = work.tile([1, GP], F32, name="gbuf", tag="gbuf", bufs=1)
                nc.scalar.activation(gbuf[:, :], psum[32:33, :], ACT.Copy)
                nc.vector.tensor_add(gbuf[:, :], psum[0:1, :], gbuf[:, :])
                nc.vector.tensor_add(gbuf[:, :], gbuf[:, :], bvec_tiles[l][:, :])
                # nonlinearities: [i(256) | f(256) | g(256) | o(256)]
                sgif = work.tile([1, 512], F32, name="sgif", tag="sgif", bufs=1)
                nc.scalar.activation(sgif[:, :], gbuf[:, 0:512], ACT.Sigmoid)
                tg = work.tile([1, SL], F32, name="tg", tag="tg", bufs=1)
                nc.scalar.activation(tg[:, :], gbuf[:, 512:768], ACT.Tanh)
                so = work.tile([1, SL], F32, name="so", tag="so", bufs=1)
                nc.scalar.activation(so[:, :], gbuf[:, 768:1024], ACT.Sigmoid)
                # c2 = sig(f)*c + sig(i)*tanh(g)
                t1 = work.tile([1, SL], F32, name="t1", tag="t1", bufs=1)
                nc.vector.tensor_mul(t1[:, :], sgif[:, 256:512], cvec_tiles[l][:, :])
                t2 = work.tile([1, SL], F32, name="t2", tag="t2", bufs=1)
                nc.vector.tensor_mul(t2[:, :], sgif[:, 0:256], tg[:, :])
                c2 = work.tile([1, SL], F32, name="c2", tag="c2", bufs=1)
                nc.vector.tensor_add(c2[:, :], t1[:, :], t2[:, :])
                # h2 = sig(o)*tanh(c2)
                tc2 = work.tile([1, SL], F32, name="tc2", tag="tc2", bufs=1)
                nc.scalar.activation(tc2[:, :], c2[:, :], ACT.Tanh)
                h2 = work.tile([1, SL], F32, name=f"h2_{l}")
                nc.vector.tensor_mul(h2[:, :], so[:, :], tc2[:, :])
                h_tiles[l] = h2
                # outputs
                nc.gpsimd.dma_start(outh_out[l - 1: l, :], h2[:, :])
                nc.gpsimd.dma_start(outc_out[l - 1: l, :], c2[:, :])
                # next-layer input slice: s = h_l (+ h_{l-1} residual for l >= 2)
                s = work.tile([1, SL], F32, name=f"s_{l}")
                if 2 <= l <= L - 1:
                    nc.vector.tensor_add(s[:, :], h2[:, :], h_tiles[l - 1][:, :])
                else:
                    nc.vector.tensor_copy(s[:, :], h2[:, :])
                # AllGather s -> full [1, 2048] as 16x128 rows
                agin = dram.tile([1, SL], F32, name=f"agin{l}")
                agout = dram.tile([16, 128], F32, name=f"agout{l}")
                dram_tiles[f"agout{l + 1}"] = agout
                nc.scalar.dma_start(agin[:], s[:, :])
                nc.gpsimd.collective_compute(
                    "AllGather",
                    mybir.AluOpType.bypass,
                    replica_groups=[list(range(NCORES))],
                    ins=[agin[:].opt()],
                    outs=[agout[:].opt()],
                )
                x16 = work.tile([16, 128], F32, name="x16", tag="x16", bufs=1)
                nc.scalar.dma_start(x16[:], agout[:])
                # transpose to [128, 16] and split into bf16 hi/lo columns
                psT = tps.tile([128, 16], F32, name="psT", tag="psT")
                nc.tensor.transpose(psT[:, :], x16[:, :], ident_sb[:, :])
                xc = work.tile([128, 2 * KT], BF, name=f"xcol_{l + 1}", tag="xcol", bufs=3)
                xf = work.tile([128, KT], F32, name="xf", tag="xf", bufs=1)
                nc.scalar.activation(xc[:, 0: 2 * KT: 2], psT[:, :], ACT.Copy)   # hi (even)
                nc.scalar.activation(xf[:, :], xc[:, 0: 2 * KT: 2], ACT.Copy)    # hi -> f32
                xlo = work.tile([128, KT], F32, name="xlo", tag="xlo", bufs=1)
                nc.vector.tensor_sub(xlo[:, :], psT[:, :], xf[:, :])
                nc.scalar.activation(xc[:, 1: 2 * KT: 2], xlo[:, :], ACT.Copy)   # lo (odd)
                xcol_tiles[l + 1] = xc

            # ================= main pipeline =================
            def load_chunk(kind, l, c):
                wt = wpool.tile([128, CH_COLS], BF, name=f"{kind}{l}_sb{c}", tag="w")
                nc.sync.dma_start(wt[:], w_ins[(kind, l, c)][:, :])
                return wt

            def load_matrix(kind, l):
                return [load_chunk(kind, l, c) for c in range(NCHUNK)]

            psums = {}

            def new_psum(l):
                ps = gps.tile([64, 1024], F32, name=f"ps{l}", tag="gps")
                psums[l] = ps
                return ps

            # PE order per layer: [AG_{l-1} lands] -> W_ih @ x_l (weights long
            # resident) -> W_hh @ h_{l+1} (riding the DMA stream; fills the PE
            # while the next AllGather is in flight).
            new_psum(1)
            issue_matrix(psums[1], load_matrix("whh", 1), hcol_pair(1), first=True, last=False)
            for ns in (0, 1):
                ps = psums[1][:, ns * 512: (ns + 1) * 512]
                rhs_hi = wih1_sb[0:2, ns * 512: (ns + 1) * 512]
                rhs_lo = wih1_sb[0:2, GP + ns * 512: GP + (ns + 1) * 512]
                mm_quad(ps, x12_sb[0:2, 0:1], x12_sb[0:2, 1:2], rhs_hi, rhs_lo, False, True)
            new_psum(2)
            issue_matrix(psums[2], load_matrix("whh", 2), hcol_pair(2), first=True, last=False)

            for l in range(2, L + 1):
                # previous layer cell + AllGather + x-column rebuild
                cell_and_handoff(l - 1, psums[l - 1])
                if l < L:
                    # chain-dependent W_ih @ x_l, then next layer's W_hh @ h
                    # (chain-independent, fills the PE during the next AllGather)
                    issue_matrix(psums[l], load_matrix("wih", l), xcol_pair(l), first=False, last=True)
                    new_psum(l + 1)
                    if l + 1 == L:
                        # Final layer: interleave the two matrices' chunk DMAs so
                        # the last weight bytes land as early as possible.
                        whhL, wihL = [], []
                        for c in range(NCHUNK):
                            whhL.append(load_chunk("whh", L, c))
                            wihL.append(load_chunk("wih", L, c))
                        issue_matrix(psums[L], whhL, hcol_pair(L), first=True, last=False)
                    else:
                        issue_matrix(psums[l + 1], load_matrix("whh", l + 1), hcol_pair(l + 1),
                                     first=True, last=False)
                else:
                    junk = gps.tile([64, 1024], F32, name="junk", tag="gps")
                    lx = hcol_pair(L)
                    for w in range(24):
                        hi, lo = lx(w % KT)
                        nc.tensor.matmul(junk[0:1, 0:512], hi,
                                         whhL[0][:, 0:512], start=w == 0, stop=w == 23)
                    issue_matrix(psums[L], wihL, xcol_pair(L), first=False, last=True)

            # final layer cell + gather of h7
            cell_and_handoff(L, psums[L])

            # ---- output projection + softmax (every core computes it) ----
            # probs tolerates bf16: single-precision x and W, 16 matmuls into
            # one PSUM row, then softmax2(z) = sigmoid(+/-(z0-z1)).
            xc7 = xcol_tiles[L + 1]
            psP = gps.tile([1, 2], F32, name="psP", tag="psP", bufs=1)
            for t in range(KT):
                nc.tensor.matmul(psP[0:1, :], xc7[:, 2 * t: 2 * t + 1],
                                 woutT_sb[:, 4 * t: 4 * t + 2],
                                 start=t == 0, stop=t == KT - 1)
            z = work.tile([1, 2], F32, name="z")
            nc.vector.tensor_add(z[:, :], psP[0:1, :], bout_sb[:, :])
            dz = work.tile([1, 1], F32, name="dz")
            nc.vector.tensor_sub(dz[:, :], z[:, 0:1], z[:, 1:2])
            dd = work.tile([1, 2], F32, name="dd")
            nc.vector.tensor_scalar_mul(dd[:, :], npn_sb[:, :], dz[:, :])
            pr = work.tile([1, 2], F32, name="pr")
            nc.scalar.activation(pr[:, :], dd[:, :], ACT.Sigmoid)
            nc.scalar.dma_start(probs_out[:, :], pr[:, :])

    nc.finalize()
    return nc


def _split_bf16(a):
    hi = a.astype(BF16)
    lo = (a - hi.astype(np.float32)).astype(BF16)
    return hi, lo


def _pack_wt(w_shard_t):
    """[2048, 1024] fp32 W^T shard -> [128, 32768] bf16 packed chunk
    (col = t*2048 + half*1024 + n)."""
    hi, lo = _split_bf16(np.ascontiguousarray(w_shard_t))
    arr = np.stack([hi.reshape(KT, 128, GP), lo.reshape(KT, 128, GP)], axis=2)
    return np.ascontiguousarray(arr.transpose(1, 0, 2, 3)).reshape(128, CHUNK_COLS)


def _hcol_pack(h):
    """[2048] fp32 -> [128, 32] bf16 (hi even cols, lo odd cols)."""
    hi, lo = _split_bf16(h)
    out = np.empty((128, 2 * KT), dtype=BF16)
    out[:, 0::2] = hi.reshape(KT, 128).T
    out[:, 1::2] = lo.reshape(KT, 128).T
    return out


_ROWS = [
    np.concatenate([np.arange(q * H + j * SL, q * H + (j + 1) * SL) for q in range(4)])
    for j in range(NCORES)
]


def kernel(x, h1, c1, h2, c2, h3, c3, h4, c4, h5, c5, h6, c6, h7, c7,
           Wih1, Whh1, bih1, bhh1, Wih, Whh, bih, bhh, Wout, bout):
    global LAST_RESULT
    from concourse.bass_utils import run_bass_kernel_spmd

    x = np.asarray(x, dtype=np.float32)
    hs = [np.asarray(a, dtype=np.float32) for a in (h1, h2, h3, h4, h5, h6, h7)]
    cs = [np.asarray(a, dtype=np.float32) for a in (c1, c2, c3, c4, c5, c6, c7)]
    Wih1 = np.asarray(Wih1, dtype=np.float32)
    Whh1 = np.asarray(Whh1, dtype=np.float32)
    Wih = np.asarray(Wih, dtype=np.float32)
    Whh = np.asarray(Whh, dtype=np.float32)
    bsum = [np.asarray(bih1, np.float32) + np.asarray(bhh1, np.float32)] + [
        np.asarray(bih, np.float32)[k] + np.asarray(bhh, np.float32)[k] for k in range(6)
    ]
    Wout = np.asarray(Wout, dtype=np.float32)
    bout = np.asarray(bout, dtype=np.float32).reshape(1, 2)

    ident = np.eye(16, dtype=np.float32)
    xhi, xlo = _split_bf16(x)
    x12 = np.stack([xhi, xlo], axis=1)  # [2, 2] (hi col 0, lo col 1)

    in_maps = []
    for j in range(NCORES):
        rows = _ROWS[j]
        m = {"ident": ident, "x12": x12, "bout": bout,
             "npn": np.array([[1.0, -1.0]], dtype=np.float32)}
        ncol = CHUNK_COLS // NCHUNK
        for l in range(1, L + 1):
            Wh = Whh1 if l == 1 else Whh[l - 2]
            p = _pack_wt(Wh[rows, :].T)
            for c in range(NCHUNK):
                m[f"whh{l}c{c}"] = np.ascontiguousarray(p[:, c * ncol:(c + 1) * ncol])
            if l >= 2:
                p = _pack_wt(Wih[l - 2][rows, :].T)
                for c in range(NCHUNK):
                    m[f"wih{l}c{c}"] = np.ascontiguousarray(p[:, c * ncol:(c + 1) * ncol])
        w1hi, w1lo = _split_bf16(np.ascontiguousarray(Wih1[rows, :].T))  # [2, 1024]
        m["wih1"] = np.concatenate([w1hi, w1lo], axis=1)  # [2, 2048]
        m["hcols"] = np.concatenate([_hcol_pack(hs[l][0]) for l in range(L)], axis=1)
        m["cvec"] = np.concatenate([cs[l][0, rows[:SL]] for l in range(L)]).reshape(1, -1)
        m["bvec"] = np.concatenate([bsum[l][rows] for l in range(L)]).reshape(1, -1)
        wthi, wtlo = _split_bf16(np.ascontiguousarray(Wout.T))  # [2048, 2]
        wpack = np.stack([wthi.reshape(KT, 128, 2), wtlo.reshape(KT, 128, 2)], axis=2)
        m["woutT"] = np.ascontiguousarray(wpack.transpose(1, 0, 2, 3)).reshape(128, KT * 4)
        in_maps.append(m)

    nc = _build_program()
    import os
    trace = bool(os.environ.get("BASS_TRACE"))
    res = run_bass_kernel_spmd(nc, in_maps, list(range(NCORES)), trace=trace)
    LAST_RESULT = res

    probs = np.asarray(res.results[0]["probs"], dtype=np.float32)
    out = [probs]
    for l in range(L):
        hv = np.concatenate([res.results[j]["outh"][l] for j in range(NCORES)]).reshape(1, H)
        cv = np.concatenate([res.results[j]["outc"][l] for j in range(NCORES)]).reshape(1, H)
        out.append(hv.astype(np.float32))
        out.append(cv.astype(np.float32))
    return tuple(out)
